# revision 1
# baseline (speedup 1.0000x reference)
"""GCAModule forward as a Bass/Tile kernel on 8 Trainium2 NeuronCores.

Sharding: data-parallel over batch N=4, 2 cores per sample. Within a
sample, the attention "p" axis (the 32x32 positions of the downsampled
grid) is split by grid rows with overlap + one fake row so that both
parities see an identical instruction stream:
  parity 0: grid rows i in [-1, 17)  (i=-1 fake, masked post-softmax)
  parity 1: grid rows i in [15, 33)  (i=32 fake, masked post-softmax)
Each core owns deconv output rows y in [32*par, 32*par+32), which land
at the SAME local rows r in [3, 35) of the padded scatter buffer for
both parities -> fully static addressing (no per-core branches).

Per-core pipeline (matmuls bf16, fp32 accumulation):
  1. gconv 1x1 (256->128) at reflect-padded downsampled positions
     -> g_pad 34x34 (q side) + a 20-row slice (p side, host-shifted).
  2. Row norms via ones-matmul over g^2 + 3x3 box sums -> f[q] =
     scale[q]/max(norm_q, eps), replicated to 128 partitions.
  3. Scaled patches phat_j = window_j(g_pad) * f (9 windows).
  4. X^T[p, q] = sum_j <wp_j[:,p], phat_j[:,q]> via 9 matmul chunks
     + a 10th identity-chunk adding the host-built diagonal penalty
     band -> PSUM holds the full softmax argument.
  5. Softmax over q (free axis) straight from PSUM; fake-p columns
     zeroed via a per-core 0/1 mask folded into 1/sum.
  6. PE-transpose gca^T -> gca[q, p].
  7. Deconv: 16 (kh,kw) taps; alpha-patch matrices A[q, o] by PE-
     transposing staged contiguous views of padded alpha; 8 q-chunk
     matmuls per tap; scatter-add into ploc[128, 38, 66].
  8. Static crop rows [3,35) cols [1,65), oconv 1x1 (x 1/4 folded into
     weights), BN partial sums, 1KB AllReduce for global stats,
     normalize + residual, DMA out [128, 32*64].
Host: prepares per-core inputs (slice/pad/cast only) and stitches the
8 x [128, 2048] outputs into (4, 128, 64, 64).
"""

import numpy as np
import ml_dtypes

import concourse.bass as bass
import concourse.bacc as bacc
import concourse.mybir as mybir
import concourse.tile as tile
from concourse.bass_utils import run_bass_kernel_spmd

F32 = mybir.dt.float32
BF16 = mybir.dt.bfloat16
NPBF = ml_dtypes.bfloat16
AX = mybir.AxisListType.X
ALU = mybir.AluOpType
ACT = mybir.ActivationFunctionType

N_CORES = 8
PENALTY = -10000.0
EPS = 1e-4
BN_EPS = 1e-5
PTILES = (128, 128, 128, 128, 64)  # p tiles per core (576 total)
P_CORE = 576
NI = 18          # local grid rows per core (incl. 1 fake)
NQC = 8          # q chunks of 128 (q = 1024)
OWN_PIX = 32 * 64


def build_program(debug: bool = False, use_cc: bool = True, stages: int = 99):
    nc = bacc.Bacc("TRN2", target_bir_lowering=False, debug=False)

    d_imgq = nc.dram_tensor("imgq", [2, 128, 1156], BF16, kind="ExternalInput")
    d_imgp = nc.dram_tensor("imgp", [2, 128, 680], BF16, kind="ExternalInput")
    d_gwT = nc.dram_tensor("gwT", [2, 128, 128], BF16, kind="ExternalInput")
    d_gb = nc.dram_tensor("gb", [128, 1], F32, kind="ExternalInput")
    d_alphap = nc.dram_tensor("alphap", [128, 66, 66], BF16, kind="ExternalInput")
    d_scalev = nc.dram_tensor("scalev", [1, 1024], F32, kind="ExternalInput")
    d_penb = nc.dram_tensor("penb", [5, 128, 1024], BF16, kind="ExternalInput")
    d_pmask = nc.dram_tensor("pmask", [128, 5], F32, kind="ExternalInput")
    d_identb = nc.dram_tensor("identb", [128, 128], BF16, kind="ExternalInput")
    d_aown = nc.dram_tensor("aown", [128, 2048], F32, kind="ExternalInput")
    d_ocwT = nc.dram_tensor("ocwT", [128, 128], BF16, kind="ExternalInput")
    d_bng = nc.dram_tensor("bng", [128, 1], F32, kind="ExternalInput")
    d_bnb = nc.dram_tensor("bnb", [128, 1], F32, kind="ExternalInput")

    d_out = nc.dram_tensor("out_own", [128, 2048], F32, kind="ExternalOutput")
    dbg = {}
    if debug:
        dbg["g_q"] = nc.dram_tensor("dbg_g_q", [128, 1156], F32, kind="ExternalOutput")
        dbg["f_row"] = nc.dram_tensor("dbg_f_row", [1, 1024], F32, kind="ExternalOutput")
        dbg["X0"] = nc.dram_tensor("dbg_X0", [128, 1024], F32, kind="ExternalOutput")
        dbg["gcaT"] = nc.dram_tensor("dbg_gcaT", [128, 5, 1024], BF16, kind="ExternalOutput")
        dbg["gca"] = nc.dram_tensor("dbg_gca", [128, 8, P_CORE], BF16, kind="ExternalOutput")
        dbg["ploc"] = nc.dram_tensor("dbg_ploc", [128, 38, 66], F32, kind="ExternalOutput")
        dbg["y"] = nc.dram_tensor("dbg_y", [128, 2048], F32, kind="ExternalOutput")
        dbg["stats"] = nc.dram_tensor("dbg_stats", [128, 2], F32, kind="ExternalOutput")

    with tile.TileContext(nc) as tc:
        with (
            tc.tile_pool(name="singles", bufs=1) as singles,
            tc.tile_pool(name="work", bufs=2) as work,
            tc.tile_pool(name="small", bufs=4) as small,
            tc.tile_pool(name="apool", bufs=3) as apool,
            tc.tile_pool(name="dram", bufs=1, space="DRAM") as dram,
            tc.tile_pool(name="psA", bufs=2, space="PSUM") as psA,
            tc.tile_pool(name="psB", bufs=2, space="PSUM") as psB,
        ):
            # ---------------- load inputs ----------------
            imgq = singles.tile([128, 2, 1156], BF16)
            imgp = singles.tile([128, 2, 680], BF16)
            for ch in range(2):
                nc.sync.dma_start(imgq[:, ch], d_imgq[ch])
                nc.sync.dma_start(imgp[:, ch], d_imgp[ch])
            gwT = singles.tile([128, 2, 128], BF16)
            for ch in range(2):
                nc.sync.dma_start(gwT[:, ch], d_gwT[ch])
            gb = singles.tile([128, 1], F32)
            nc.sync.dma_start(gb, d_gb[:])
            alphap = singles.tile([128, 66, 66], BF16)
            nc.sync.dma_start(alphap, d_alphap[:])
            scalev = singles.tile([1, 1024], F32)
            nc.sync.dma_start(scalev, d_scalev[:])
            penb = singles.tile([128, 5, 1024], BF16)
            for t in range(5):
                nc.sync.dma_start(penb[:, t], d_penb[t])
            pmask = singles.tile([128, 5], F32)
            nc.sync.dma_start(pmask, d_pmask[:])
            identb = singles.tile([128, 128], BF16)
            nc.sync.dma_start(identb, d_identb[:])
            aown = singles.tile([128, 2048], F32)
            nc.sync.dma_start(aown, d_aown[:])
            ocwT = singles.tile([128, 128], BF16)
            nc.sync.dma_start(ocwT, d_ocwT[:])
            bng = singles.tile([128, 1], F32)
            nc.sync.dma_start(bng, d_bng[:])
            bnb = singles.tile([128, 1], F32)
            nc.sync.dma_start(bnb, d_bnb[:])

            # ---------------- gconv ----------------
            # q-side: g over the full 34x34 padded grid
            pg1 = psA.tile([128, 1024], F32, tag="ps2bank")
            pg2 = psB.tile([128, 512], F32, tag="psB")
            for ch in range(2):
                nc.tensor.matmul(pg1[:, 0:512], gwT[:, ch], imgq[:, ch, 0:512],
                                 start=(ch == 0), stop=(ch == 1))
                nc.tensor.matmul(pg1[:, 512:1024], gwT[:, ch], imgq[:, ch, 512:1024],
                                 start=(ch == 0), stop=(ch == 1))
                nc.tensor.matmul(pg2[:, 0:132], gwT[:, ch], imgq[:, ch, 1024:1156],
                                 start=(ch == 0), stop=(ch == 1))
            g_q = singles.tile([128, 1156], F32)
            nc.scalar.add(g_q[:, 0:1024], pg1[:], gb)
            nc.scalar.add(g_q[:, 1024:1156], pg2[:, 0:132], gb)
            g_qb = singles.tile([128, 1156], BF16)
            nc.vector.tensor_copy(g_qb, g_q)
            if debug:
                nc.sync.dma_start(dbg["g_q"][:], g_q)

            # p-side: 20 padded rows (host supplies the parity-shifted slice)
            pgp = psB.tile([128, 680], F32, tag="psB")
            for ch in range(2):
                nc.tensor.matmul(pgp[:, 0:512], gwT[:, ch], imgp[:, ch, 0:512],
                                 start=(ch == 0), stop=(ch == 1))
                nc.tensor.matmul(pgp[:, 512:680], gwT[:, ch], imgp[:, ch, 512:680],
                                 start=(ch == 0), stop=(ch == 1))
            g_pb = singles.tile([128, 680], BF16)
            nc.scalar.activation(g_pb, pgp[:], ACT.Identity, bias=gb, scale=1.0)

            # ---------------- norms -> f row -> F broadcast ----------------
            g2 = singles.tile([128, 1156], F32)
            nc.vector.tensor_mul(g2, g_q, g_q)
            ones = singles.tile([128, 1], F32)
            nc.vector.memset(ones, 1.0)
            pe1 = psB.tile([1, 1024], F32, tag="psB")
            pe2 = psB.tile([1, 512], F32, tag="psB")
            nc.tensor.matmul(pe1[:, 0:512], ones, g2[:, 0:512], start=True, stop=True)
            nc.tensor.matmul(pe1[:, 512:1024], ones, g2[:, 512:1024], start=True, stop=True)
            nc.tensor.matmul(pe2[:, 0:132], ones, g2[:, 1024:1156], start=True, stop=True)
            e_sb = singles.tile([1, 34, 34], F32)
            e_flat = e_sb.rearrange("p a b -> p (a b)")
            nc.vector.tensor_copy(e_flat[:, 0:1024], pe1[:])
            nc.vector.tensor_copy(e_flat[:, 1024:1156], pe2[:, 0:132])
            rsum = singles.tile([1, 34, 32], F32)
            nc.vector.tensor_tensor(rsum, e_sb[:, :, 0:32], e_sb[:, :, 1:33], op=ALU.add)
            nc.vector.tensor_tensor(rsum, rsum, e_sb[:, :, 2:34], op=ALU.add)
            n2 = singles.tile([1, 32, 32], F32)
            nc.vector.tensor_tensor(n2, rsum[:, 0:32], rsum[:, 1:33], op=ALU.add)
            nc.vector.tensor_tensor(n2, n2, rsum[:, 2:34], op=ALU.add)
            n2f = n2.rearrange("p a b -> p (a b)")
            f_row = singles.tile([1, 1024], F32)
            nc.scalar.sqrt(f_row, n2f)
            nc.vector.tensor_scalar_max(f_row, f_row, EPS)
            nc.vector.reciprocal(f_row, f_row)
            nc.vector.tensor_mul(f_row, f_row, scalev)
            if debug:
                nc.sync.dma_start(dbg["f_row"][:], f_row)
            f_dram = dram.tile([1, 1024], F32)
            nc.sync.dma_start(f_dram, f_row)
            F_rep = singles.tile([128, 1024], F32)
            nc.gpsimd.dma_start(F_rep, f_dram[:].to_broadcast((128, 1024)))

            if stages < 2:
                nc.sync.dma_start(d_out[:], aown)
                nc.finalize_hint = None
            # ---------------- staged patch operands ----------------
            gp3 = g_pb.rearrange("c (a b) -> c a b", a=20)
            gq3 = g_qb.rearrange("c (a b) -> c a b", a=34)
            # stationary windows wp_j [128c, 576p] (contiguous for LDWEIGHTS)
            wp = singles.tile([128, 9, P_CORE], BF16)
            # moving scaled patches phat_j [128c, 1024q] = window_j(g_q) * f
            phat = singles.tile([128, 9, 1024], BF16)
            for kj in range(3):
                for lj in range(3):
                    j = 3 * kj + lj
                    nc.any.tensor_copy(
                        wp[:, j].rearrange("c (a b) -> c a b", a=NI),
                        gp3[:, kj:kj + NI, lj:lj + 32])
                    nc.vector.tensor_tensor(
                        phat[:, j].rearrange("c (a b) -> c a b", a=32),
                        gq3[:, kj:kj + 32, lj:lj + 32],
                        F_rep.rearrange("c (a b) -> c a b", a=32),
                        op=ALU.mult)

            # ---------------- X^T + softmax per p-tile ----------------
            gcaT = singles.tile([128, 5, 1024], BF16)
            if debug:
                nc.vector.memset(gcaT, 0.0)

            for t, sz in enumerate(PTILES):
                pS = psA.tile([128, 1024], F32, tag="ps2bank")
                for j in range(9):
                    lhsT = wp[:, j, 128 * t:128 * t + sz]
                    for h in range(2):
                        nc.tensor.matmul(
                            pS[:sz, 512 * h:512 * h + 512], lhsT,
                            phat[:, j, 512 * h:512 * h + 512],
                            start=(j == 0), stop=False, skip_group_check=True)
                # diagonal penalty chunk: identity x penalty band
                for h in range(2):
                    nc.tensor.matmul(
                        pS[:sz, 512 * h:512 * h + 512], identb[:, :sz],
                        penb[:, t, 512 * h:512 * h + 512],
                        start=False, stop=True, skip_group_check=True)
                if debug and t == 0:
                    xdbg = work.tile([128, 1024], F32, tag="X")
                    nc.vector.tensor_copy(xdbg[:sz], pS[:sz])
                    nc.sync.dma_start(dbg["X0"][:], xdbg)
                negmax = small.tile([128, 1], F32, tag="negmax")
                nc.vector.reduce_max(negmax[:sz], pS[:sz], axis=AX, negate=True)
                E = work.tile([128, 1024], BF16, tag="E")
                ssum = small.tile([128, 1], F32, tag="ssum")
                nc.scalar.activation(E[:sz], pS[:sz], ACT.Exp, bias=negmax[:sz],
                                     scale=1.0, accum_out=ssum[:sz])
                rinv = small.tile([128, 1], F32, tag="rinv")
                nc.vector.reciprocal(rinv[:sz], ssum[:sz])
                # zero fake-p columns by folding the 0/1 mask into 1/sum
                nc.vector.tensor_mul(rinv[:sz], rinv[:sz], pmask[:sz, t:t + 1])
                nc.vector.tensor_scalar_mul(gcaT[:sz, t, :], E[:sz], rinv[:sz])
            if debug:
                nc.sync.dma_start(dbg["gcaT"][:], gcaT)

            # ---------------- transpose gca^T -> gca[q, p] ----------------
            gca = singles.tile([128, 8, P_CORE], BF16)
            for qc in range(NQC):
                for t, sz in enumerate(PTILES):
                    ptr = psB.tile([128, 128], BF16, tag="psB")
                    nc.tensor.transpose(ptr[:, :sz],
                                        gcaT[:sz, t, 128 * qc:128 * qc + 128],
                                        identb[:sz, :sz])
                    nc.any.tensor_copy(gca[:, qc, 128 * t:128 * t + sz], ptr[:, :sz])
            if debug:
                nc.sync.dma_start(dbg["gca"][:], gca)

            # ---------------- deconv: 16 taps ----------------
            ploc = singles.tile([128, 38, 66], F32)
            nc.vector.memset(ploc, 0.0)
            for kh in range(4):
                for kw in range(4):
                    # stage A^T_khkw [o, q] contiguous (stationary needs 1 free dim)
                    at = apool.tile([128, 1024], BF16, tag="at")
                    nc.any.tensor_copy(
                        at.rearrange("c (a b) -> c a b", a=32),
                        alphap[:, kh:kh + 63:2, kw:kw + 63:2])
                    pT = psA.tile([128, 1024], F32, tag="ps2bank")
                    for qc in range(NQC):
                        pA = psB.tile([128, 128], BF16, tag="psB")
                        nc.tensor.transpose(pA, at[:, 128 * qc:128 * qc + 128], identb)
                        a_sb = apool.tile([128, 128], BF16, tag="a_sb")
                        nc.any.tensor_copy(a_sb, pA)
                        nc.tensor.matmul(pT[:, 0:512], a_sb, gca[:, qc, 0:512],
                                         start=(qc == 0), stop=(qc == NQC - 1),
                                         skip_group_check=True)
                        nc.tensor.matmul(pT[:, 512:P_CORE], a_sb, gca[:, qc, 512:P_CORE],
                                         start=(qc == 0), stop=(qc == NQC - 1),
                                         skip_group_check=True)
                    tgt = ploc[:, kh:kh + 35:2, kw:kw + 63:2]
                    src = pT[:, 0:P_CORE].rearrange("p (a b) -> p a b", a=NI)
                    nc.vector.tensor_tensor(tgt, tgt, src, op=ALU.add)
            if debug:
                nc.sync.dma_start(dbg["ploc"][:], ploc)

            # ---------------- crop owned rows + oconv + BN ----------------
            prop = singles.tile([128, 2048], BF16)
            prop3 = prop.rearrange("c (a b) -> c a b", a=32)
            nc.vector.tensor_copy(prop3, ploc[:, 3:35, 1:65])
            py = psA.tile([128, 1024], F32, tag="ps2bank")
            py2 = psB.tile([128, 1024], F32, tag="psB")
            for h, pt in enumerate((py, py2)):
                for s in range(2):
                    nc.tensor.matmul(pt[:, 512 * s:512 * s + 512], ocwT,
                                     prop[:, 1024 * h + 512 * s:1024 * h + 512 * s + 512],
                                     start=True, stop=True)
            y = singles.tile([128, 2048], F32)
            nc.scalar.copy(y[:, 0:1024], py[:])
            nc.scalar.copy(y[:, 1024:2048], py2[:])
            if debug:
                nc.sync.dma_start(dbg["y"][:], y)
            y2 = singles.tile([128, 2048], F32)
            nc.vector.tensor_mul(y2, y, y)
            s1 = small.tile([128, 1], F32, tag="s1")
            s2 = small.tile([128, 1], F32, tag="s2")
            nc.vector.reduce_sum(s1, y, axis=AX)
            nc.vector.reduce_sum(s2, y2, axis=AX)
            stats = singles.tile([128, 2], F32)
            nc.vector.tensor_copy(stats[:, 0:1], s1)
            nc.vector.tensor_copy(stats[:, 1:2], s2)
            if debug:
                nc.sync.dma_start(dbg["stats"][:], stats)

            gstats = singles.tile([128, 2], F32)
            if use_cc:
                cc_in = dram.tile([128, 2], F32)
                cc_out = dram.tile([128, 2], F32, addr_space="Shared")
                nc.sync.dma_start(cc_in, stats)
                nc.gpsimd.collective_compute(
                    "AllReduce", ALU.add,
                    replica_groups=[list(range(N_CORES))],
                    ins=[cc_in[:].opt()], outs=[cc_out[:].opt()])
                nc.sync.dma_start(gstats, cc_out)
                inv_n = 1.0 / float(N_CORES * OWN_PIX)
            else:
                nc.vector.tensor_copy(gstats, stats)
                inv_n = 1.0 / float(OWN_PIX)

            mu = small.tile([128, 1], F32, tag="mu")
            nc.vector.tensor_scalar_mul(mu, gstats[:, 0:1], inv_n)
            msq = small.tile([128, 1], F32, tag="msq")
            nc.vector.tensor_scalar_mul(msq, gstats[:, 1:2], inv_n)
            var = small.tile([128, 1], F32, tag="var")
            nc.vector.tensor_mul(var, mu, mu)
            nc.vector.tensor_tensor(var, msq, var, op=ALU.subtract)
            std = small.tile([128, 1], F32, tag="std")
            epsb = small.tile([128, 1], F32, tag="epsb")
            nc.vector.memset(epsb, BN_EPS)
            nc.scalar.activation(std, var, ACT.Sqrt, bias=epsb, scale=1.0)
            nc.vector.reciprocal(std, std)
            a_sc = small.tile([128, 1], F32, tag="a_sc")
            nc.vector.tensor_mul(a_sc, bng, std)
            b_sc = small.tile([128, 1], F32, tag="b_sc")
            nc.vector.tensor_mul(b_sc, mu, a_sc)
            nc.vector.tensor_tensor(b_sc, bnb, b_sc, op=ALU.subtract)
            o_sb = singles.tile([128, 2048], F32)
            nc.vector.tensor_scalar(o_sb, y, scalar1=a_sc, scalar2=b_sc,
                                    op0=ALU.mult, op1=ALU.add)
            nc.vector.tensor_tensor(o_sb, o_sb, aown, op=ALU.add)
            nc.sync.dma_start(d_out[:], o_sb)

    nc.finalize()
    return nc


def _box3_mean(u_pad):
    s = np.zeros((u_pad.shape[0] - 2, u_pad.shape[1] - 2), np.float32)
    for a in range(3):
        for b in range(3):
            s += u_pad[a:a + s.shape[0], b:b + s.shape[1]]
    return s / np.float32(9.0)


def core_grid_rows(par):
    """Global grid row index for each of the NI local rows (may be -1/32 fake)."""
    return np.arange(NI) - 1 + 16 * par  # par0: -1..16, par1: 15..32


def make_core_inputs(img_feat, alpha_feat, unknown, gconv_w, gconv_b, oconv_w,
                     bn_gamma, bn_beta):
    """Host-side shard prep: returns list of 8 per-core input dicts."""
    img_feat = np.asarray(img_feat, np.float32)
    alpha_feat = np.asarray(alpha_feat, np.float32)
    unknown = np.asarray(unknown, np.float32)
    gconv_w = np.asarray(gconv_w, np.float32)
    gconv_b = np.asarray(gconv_b, np.float32)
    oconv_w = np.asarray(oconv_w, np.float32)
    bn_gamma = np.asarray(bn_gamma, np.float32)
    bn_beta = np.asarray(bn_beta, np.float32)

    gwT = np.ascontiguousarray(gconv_w.T).reshape(2, 128, 128).astype(NPBF)
    gb = gconv_b.reshape(128, 1).astype(np.float32)
    ocwT = np.ascontiguousarray((0.25 * oconv_w.T)).astype(NPBF)
    bng = bn_gamma.reshape(128, 1).astype(np.float32)
    bnb = bn_beta.reshape(128, 1).astype(np.float32)
    identb = np.eye(128, dtype=np.float32).astype(NPBF)

    in_maps = []
    for core in range(N_CORES):
        n, par = core // 2, core % 2
        img_ds = img_feat[n][:, ::2, ::2]
        img_pad = np.pad(img_ds, ((0, 0), (1, 1), (1, 1)), mode="reflect")
        imgq = np.ascontiguousarray(img_pad.reshape(2, 128, 1156)).astype(NPBF)
        # p-side rows: device patch at local row i_loc reads p-side rows
        # i_loc+kj; local grid row g = i_loc-1+16*par has patch rows =
        # padded rows g+kj.  So p-side row r holds padded row r-1+16*par,
        # clamped at the fake ends (content masked post-softmax).
        rows = np.clip(np.arange(20) - 1 + 16 * par, 0, 33)
        imgp_arr = img_pad[:, rows, :]
        imgp = np.ascontiguousarray(imgp_arr.reshape(2, 128, 680)).astype(NPBF)
        alphap = np.pad(alpha_feat[n], ((0, 0), (1, 1), (1, 1)),
                        mode="reflect").astype(NPBF)

        u = unknown[n, 0][::2, ::2].astype(np.float32)
        um = u.mean(dtype=np.float32)
        km = np.float32(1.0) - um
        with np.errstate(divide="ignore", invalid="ignore"):
            us = np.clip(np.sqrt(um / km), 0.1, 10.0).astype(np.float32)
            ks = np.clip(np.sqrt(km / um), 0.1, 10.0).astype(np.float32)
        u_pad = np.pad(u, ((1, 1), (1, 1)), mode="reflect")
        unk_ps = _box3_mean(u_pad).reshape(1024)
        is_unk = unk_ps > 0.0
        scalev = np.where(is_unk, us, ks).astype(np.float32).reshape(1, 1024)
        pen = (np.float32(PENALTY) * unk_ps).astype(np.float32)

        # penalty bands + fake-p mask
        penb = np.zeros((5, 128, 1024), NPBF)
        pmask = np.zeros((128, 5), np.float32)
        grows = np.arange(NI) - 1 + 16 * par          # global grid row per local
        for t, sz in enumerate(PTILES):
            pl = 128 * t + np.arange(sz)              # local p index
            gi = grows[pl // 32]
            gj = pl % 32
            real = (gi >= 0) & (gi < 32)
            pg = gi * 32 + gj
            pmask[:sz, t] = real.astype(np.float32)
            rr = np.where(real)[0]
            penb[t, rr, pg[rr]] = pen[pg[rr]].astype(NPBF)
        aown = np.ascontiguousarray(
            alpha_feat[n][:, 32 * par:32 * par + 32, :].reshape(128, 2048)
        ).astype(np.float32)

        in_maps.append(dict(
            imgq=imgq, imgp=imgp, gwT=gwT, gb=gb, alphap=alphap,
            scalev=scalev, penb=penb, pmask=pmask, identb=identb,
            aown=aown, ocwT=ocwT, bng=bng, bnb=bnb,
        ))
    return in_maps


_CACHE = {}


def _get_program(debug=False, use_cc=True):
    key = (bool(debug), bool(use_cc))
    if key not in _CACHE:
        _CACHE[key] = build_program(debug=key[0], use_cc=key[1])
    return _CACHE[key]


def kernel(img_feat, alpha_feat, unknown, gconv_w, gconv_b, oconv_w,
           bn_gamma, bn_beta, _debug=False, _trace=False, _use_cc=True):
    in_maps = make_core_inputs(img_feat, alpha_feat, unknown, gconv_w, gconv_b,
                               oconv_w, bn_gamma, bn_beta)
    nc = _get_program(debug=_debug, use_cc=_use_cc)
    res = run_bass_kernel_spmd(nc, in_maps, core_ids=list(range(N_CORES)),
                               trace=_trace)
    out = np.zeros((4, 128, 64, 64), np.float32)
    for core in range(N_CORES):
        n, par = core // 2, core % 2
        out[n, :, 32 * par:32 * par + 32, :] = (
            res.results[core]["out_own"].reshape(128, 32, 64))
    kernel.last_result = res
    return out

